# revision 39
# baseline (speedup 1.0000x reference)
import sys

sys.path.insert(0, "/opt/trn_rl_repo")

from contextlib import ExitStack

import numpy as np

import concourse.bass as bass  # noqa: F401
import concourse.bacc as bacc
import concourse.tile as tile
from concourse import mybir
from concourse.bass_utils import run_bass_kernel_spmd
from concourse.masks import make_identity

F32 = mybir.dt.float32
BF16 = mybir.dt.bfloat16
FP8 = mybir.dt.float8e4
DR = mybir.MatmulPerfMode.DoubleRow
AX = mybir.AxisListType.X
MAX = mybir.AluOpType.max
MULT = mybir.AluOpType.mult
ADD = mybir.AluOpType.add
EXP = mybir.ActivationFunctionType.Exp
CPY = mybir.ActivationFunctionType.Copy

C = 512          # channels
HW = 4096        # spatial positions (64*64)
HID = 64         # attention hidden dim (C // 8)
MH = 2048        # spatial positions handled per core (HW / 2)
NB = 4           # channel blocks of 128
NT = 32          # spatial tiles of 128 (full HW)
EXP_SHIFT = -24.0  # constant logit shift: exact softmax, avoids fp32 overflow

_cache = {}


def _build(gp: float, gc: float):
    nc = bacc.Bacc("TRN2", target_bir_lowering=False, debug=False, num_devices=8)

    feat_d = nc.dram_tensor("feat", [C, HW], BF16, kind="ExternalInput")
    feat8_d = nc.dram_tensor("feat8", [C, HW], FP8, kind="ExternalInput")
    featT8_d = nc.dram_tensor("featT8", [HW, C], FP8, kind="ExternalInput")
    wqt_d = nc.dram_tensor("wqt", [C, HID], BF16, kind="ExternalInput")
    wkt_d = nc.dram_tensor("wkt", [C, HID], BF16, kind="ExternalInput")
    wvt8_d = nc.dram_tensor("wvt8", [C, C], FP8, kind="ExternalInput")
    # m-major output: o[m, c] = gp*pam[m, c] + gc*cam[m, c]; host adds 2*x.
    o_d = nc.dram_tensor("o", [MH, C], BF16, kind="ExternalOutput")

    feat_b = feat_d.ap().rearrange("(cb p) n -> p cb n", p=128)
    feat8_b = feat8_d.ap().rearrange("(cb p) n -> p cb n", p=128)
    featT8_b = featT8_d.ap().rearrange("(nt p) c -> p nt c", p=128)

    with tile.TileContext(nc) as tc, ExitStack() as S:
        A = S.enter_context(tc.tile_pool(name="pA", bufs=1))

        wqt = A.tile([128, NB, HID], BF16)
        wkt = A.tile([128, NB, HID], BF16)
        wvt8 = A.tile([128, NB, C], FP8)
        # weights + first feat half lead the scalar queue so the q
        # projection (first PE work) starts as early as possible; the
        # fp8 bulk loads follow behind.
        nc.scalar.dma_start(wqt, wqt_d.ap().rearrange("(cb p) o -> p cb o", p=128))
        nc.scalar.dma_start(wkt, wkt_d.ap().rearrange("(cb p) o -> p cb o", p=128))
        fc_sb = A.tile([128, NB, HW], BF16)
        nc.scalar.dma_start(fc_sb[:, :, 0:MH], feat_b[:, :, 0:MH])
        nc.sync.dma_start(fc_sb[:, :, MH:HW], feat_b[:, :, MH:HW])
        nc.scalar.dma_start(wvt8, wvt8_d.ap().rearrange("(cb p) o -> p cb o", p=128))
        idf = A.tile([128, 128], BF16)
        make_identity(nc, idf)
        shift = A.tile([128, 1], F32)
        nc.vector.memset(shift, EXP_SHIFT)

        # fp8 copies: c-major (for v-proj / CAM apply) and n-major (for Gram)
        fc8 = A.tile([128, NB, HW], FP8)
        nc.sync.dma_start(fc8, feat8_b)
        fT8 = A.tile([128, NT, C], FP8)
        nc.scalar.dma_start(fT8, featT8_b)

        # k and q with duplicate copies at partitions 64:128 (for PE row tiling)
        kq_k = A.tile([128, HW], BF16)
        kq_q = A.tile([128, MH], BF16)
        vT = A.tile([128, NT, C + 2], BF16)  # cols 0:2 = ones (softmax Z trick)
        nc.vector.memset(vT[:, :, 0:2], 1.0)
        ET8 = A.tile([128, NB, C], FP8)      # ET[d, c] = E[c, d] * gc / Z_c
        cam_sb = A.tile([128, MH // 128, C], BF16)  # gc * cam^T[m, c]

        # ---------- S1: q, k projections + dual copies ----------
        with ExitStack() as S1:
            psQ = S1.enter_context(tc.tile_pool(name="psQ", bufs=2, space="PSUM"))
            for nh in range(MH // 512):
                pq = psQ.tile([64, 512], F32, tag="pq")
                for cb in range(NB):
                    nc.tensor.matmul(
                        pq, wqt[:, cb, :], fc_sb[:, cb, nh * 512:(nh + 1) * 512],
                        start=(cb == 0), stop=(cb == NB - 1),
                    )
                nc.vector.tensor_copy(kq_q[0:64, nh * 512:(nh + 1) * 512], pq)
            for nh in range(HW // 512):
                pk = psQ.tile([64, 512], F32, tag="pk")
                for cb in range(NB):
                    nc.tensor.matmul(
                        pk, wkt[:, cb, :], fc_sb[:, cb, nh * 512:(nh + 1) * 512],
                        start=(cb == 0), stop=(cb == NB - 1),
                    )
                nc.vector.tensor_copy(kq_k[0:64, nh * 512:(nh + 1) * 512], pk)
            nc.sync.dma_start(kq_q[64:128, :], kq_q[0:64, :])
            nc.sync.dma_start(kq_k[64:128, :], kq_k[0:64, :])

        # ---------- S2: v projection (fp8 DR) ----------
        with ExitStack() as S2:
            psV = S2.enter_context(tc.tile_pool(name="psV", bufs=2, space="PSUM"))
            for nt in range(NT):
                pv = psV.tile([128, C], F32, tag="pv")
                for pr in range(NB // 2):
                    nc.tensor.matmul(
                        pv,
                        fc8[:, 2 * pr:2 * pr + 2, nt * 128:(nt + 1) * 128],
                        wvt8[:, 2 * pr:2 * pr + 2, :],
                        start=(pr == 0), stop=(pr == NB // 2 - 1),
                        perf_mode=DR,
                    )
                nc.vector.tensor_copy(vT[:, nt, 2:C + 2], pv)

        # ---------- S3: CAM Gram (fp8 DR) + E^T ----------
        with ExitStack() as S3:
            Dp = S3.enter_context(tc.tile_pool(name="pD", bufs=1))
            psG = S3.enter_context(tc.tile_pool(name="psG", bufs=2, space="PSUM"))
            psE = S3.enter_context(tc.tile_pool(name="psE", bufs=2, space="PSUM"))
            for ct in range(NB):
                pg = psG.tile([128, C], F32, tag="pg")
                for pr in range(NT // 2):
                    nc.tensor.matmul(
                        pg,
                        fT8[:, 2 * pr:2 * pr + 2, ct * 128:(ct + 1) * 128],
                        fT8[:, 2 * pr:2 * pr + 2, :],
                        start=(pr == 0), stop=(pr == NT // 2 - 1),
                        perf_mode=DR,
                    )
                negmax = Dp.tile([128, 1], F32, tag="negmax", bufs=2)
                nc.vector.tensor_reduce(negmax, pg, axis=AX, op=MAX, negate=True)
                sums = Dp.tile([128, 1], F32, tag="sums", bufs=2)
                E = Dp.tile([128, C], BF16, tag="E", bufs=2)
                nc.scalar.activation(E, pg, EXP, bias=negmax, accum_out=sums)
                recip = Dp.tile([128, 1], F32, tag="recip", bufs=2)
                nc.vector.reciprocal(recip, sums)
                scal = Dp.tile([128, 1], F32, tag="scal", bufs=2)
                nc.vector.tensor_scalar_mul(scal, recip, gc)
                Dg = Dp.tile([128, 128], BF16, tag="Dg", bufs=2)
                nc.vector.tensor_scalar_mul(Dg, idf, scal)
                pet = psE.tile([128, NB, 128], F32, tag="pet")
                for db in range(NB):
                    nc.tensor.matmul(
                        pet[:, db, :],
                        E[:, db * 128:(db + 1) * 128],
                        Dg,
                        start=True, stop=True,
                    )
                nc.vector.tensor_copy(ET8[:, :, ct * 128:(ct + 1) * 128], pet)

        # ---------- S4: CAM apply (fp8 DR, m-major) ----------
        with ExitStack() as S4:
            psC = S4.enter_context(tc.tile_pool(name="psC", bufs=2, space="PSUM"))
            for mt in range(MH // 128):
                pcam = psC.tile([128, C], F32, tag="pcam")
                for pr in range(NB // 2):
                    nc.tensor.matmul(
                        pcam,
                        fc8[:, 2 * pr:2 * pr + 2, mt * 128:(mt + 1) * 128],
                        ET8[:, 2 * pr:2 * pr + 2, :],
                        start=(pr == 0), stop=(pr == NB // 2 - 1),
                        perf_mode=DR,
                    )
                nc.scalar.activation(cam_sb[:, mt, :], pcam, CPY)

        # ---------- S5: PAM over 4 m-groups of 512 ----------
        with ExitStack() as S5:
            Gp = S5.enter_context(tc.tile_pool(name="pG", bufs=1))
            psL = S5.enter_context(tc.tile_pool(name="psL", bufs=2, space="PSUM"))
            psO = S5.enter_context(tc.tile_pool(name="psO", bufs=3, space="PSUM"))
            for g in range(MH // 512):
                m0 = g * 512
                sts = []
                for nt in range(NT):
                    hi = 64 * (nt % 2)  # alternate PE row-tiles T0 / T8
                    pl = psL.tile([128, 512], F32, tag="pl")
                    nc.tensor.matmul(
                        pl,
                        kq_k[hi:hi + 64, nt * 128:(nt + 1) * 128],
                        kq_q[hi:hi + 64, m0:m0 + 512],
                        start=True, stop=True,
                    )
                    st = Gp.tile([128, 512], BF16, tag="st", bufs=34)
                    nc.scalar.activation(st, pl, EXP, bias=shift)
                    sts.append(st)
                for mt in range(4):
                    pa = psO.tile([128, 258], F32, tag="pa")
                    pb = psO.tile([128, 256], F32, tag="pb")
                    for nt in range(NT):
                        lhs = sts[nt][:, mt * 128:(mt + 1) * 128]
                        nc.tensor.matmul(pa, lhs, vT[:, nt, 0:258],
                                         start=(nt == 0), stop=(nt == NT - 1))
                        nc.tensor.matmul(pb, lhs, vT[:, nt, 258:C + 2],
                                         start=(nt == 0), stop=(nt == NT - 1))
                    recip = Gp.tile([128, 1], F32, tag="recip4", bufs=2)
                    nc.vector.reciprocal(recip, pa[:, 0:1])
                    scalp = Gp.tile([128, 1], F32, tag="scalp", bufs=2)
                    nc.vector.tensor_scalar_mul(scalp, recip, gp)
                    mti = g * 4 + mt
                    o_sb = Gp.tile([128, C], BF16, tag="osb", bufs=3)
                    nc.vector.scalar_tensor_tensor(
                        o_sb[:, 0:256], pa[:, 2:258], scalp,
                        cam_sb[:, mti, 0:256], op0=MULT, op1=ADD,
                    )
                    nc.vector.scalar_tensor_tensor(
                        o_sb[:, 256:C], pb, scalp,
                        cam_sb[:, mti, 256:C], op0=MULT, op1=ADD,
                    )
                    nc.sync.dma_start(
                        o_d.ap()[mti * 128:(mti + 1) * 128, :], o_sb
                    )

    nc.finalize()
    return nc


def make_in_maps(x, Wq, Wk, Wv):
    import ml_dtypes

    bf16 = ml_dtypes.bfloat16
    fp8 = ml_dtypes.float8_e4m3
    wqt = np.ascontiguousarray(np.asarray(Wq, np.float32).T).astype(bf16)
    wkt = np.ascontiguousarray(np.asarray(Wk, np.float32).T).astype(bf16)
    wvt8 = np.ascontiguousarray(np.asarray(Wv, np.float32).T).astype(fp8)
    in_maps = []
    for core in range(8):
        b, h = divmod(core, 2)
        feat = np.asarray(x[b], np.float32).reshape(C, HW)
        # rotate columns so this core's m-half sits at columns 0:MH
        rolled = np.roll(feat, -h * MH, axis=1) if h else feat
        rolled = np.ascontiguousarray(rolled)
        in_maps.append({
            "feat": rolled.astype(bf16),
            "feat8": rolled.astype(fp8),
            "featT8": np.ascontiguousarray(rolled.T).astype(fp8),
            "wqt": wqt, "wkt": wkt, "wvt8": wvt8,
        })
    return in_maps


def kernel(x, Wq, Wk, Wv, gamma_p, gamma_c):
    x = np.asarray(x, dtype=np.float32)
    gp = float(np.asarray(gamma_p).reshape(-1)[0])
    gc = float(np.asarray(gamma_c).reshape(-1)[0])
    key = (gp, gc)
    if key not in _cache:
        _cache[key] = _build(gp, gc)
    nc = _cache[key]

    in_maps = make_in_maps(x, Wq, Wk, Wv)
    res = run_bass_kernel_spmd(nc, in_maps, core_ids=list(range(8)))

    out = np.empty((x.shape[0], C, HW), dtype=np.float32)
    for core in range(8):
        b, h = divmod(core, 2)
        # o[m, c] holds gp*pam + gc*cam for this core's m-half; add residual 2x
        o = np.asarray(res.results[core]["o"], dtype=np.float32).T
        out[b][:, h * MH:(h + 1) * MH] = o + 2.0 * x[b].reshape(C, HW)[:, h * MH:(h + 1) * MH]
    return out.reshape(x.shape[0], C, 64, 64)
